# revision 1
# baseline (speedup 1.0000x reference)
"""Trainium2 Bass kernel for NeuronInvariantDeepSetLayer (segment_reduce).

kernel(**inputs) takes FULL unsharded inputs (as in reference.setup_inputs())
and returns the full [4096, 1] float32 output.

Strategy: data-parallel over 8 NeuronCores. Segments are split 512/core
(idx is sorted, so each core's rows are a contiguous slice of x). Rows are
host-padded so that each 128-segment block starts exactly at a 128-row tile
boundary -> every core runs the IDENTICAL instruction stream (pure SPMD),
only the data differs.

Per core device pipeline (bf16 phi / f32 accumulate / f32 rho):
  - SWDGE DMA: x rows f32 HBM -> SBUF bf16 (cast in flight), 1024 rows/DMA
  - PE transpose x tiles -> xT (feat on partitions)
  - mm1: lhsT=W1 chunks (stationary), rhs=xT -> h1T psum [hid, 512 rows]
  - ACT relu psum -> SBUF bf16 (hid on partitions = per-partition bias layout)
  - mm2: lhsT=h1T chunks, rhs=W2 -> h2 psum [128 rows, 192]
  - seg reduce: sel = is_equal(idx_local, iota) one-hot [128 rows, 128 segs];
    matmul(psum_seg += sel.T @ h2) accumulated in PSUM over ~100 tiles
  - rho (tiny, f32): transpose x_sum, 2 matmuls + relu -> out [128] per block
"""

import sys
import os

sys.path.insert(0, "/opt/trn_rl_repo")

import numpy as np
import ml_dtypes

N = 400000
B = 4096
DIN = 768
DHID = 192
HPAD = 256  # hid padded to 2x128 so all weight loads are 128-col (FWL)
NCORES = 8
SPC = B // NCORES  # segments per core = 512
SBLK = 128  # segments per seg-block (psum accumulator width)
NBLK = SPC // SBLK  # 4 seg-blocks per core
P = 128
KC1 = DIN // P  # 6 k-chunks for mm1
CH = 1024  # rows per DMA chunk (8 sub-tiles)

f32 = np.float32
bf16 = ml_dtypes.bfloat16
fp8 = ml_dtypes.float8_e4m3

# mm1 in fp8e4m3 with DoubleRow (2 MACs/cell/cycle). W1 is pre-scaled by
# FP8_SCALE into fp8's normal range; the inverse is folded into W2 exactly.
USE_FP8_MM1 = False
FP8_SCALE = 32.0

# which of the 4 sub-tiles per mblock use the DMA xbar transpose
# (SBUF->SBUF, one instruction per sub-tile) instead of 6 PE transposes +
# a PSUM->SBUF copy. Offloading half keeps the SBUF AXI fabric under budget.
DMA_T_SUBTILES = (1, 3)


def _prep(x, idx):
    """Host-side sharding. Returns per-core padded shards + layout params."""
    if np.any(np.diff(idx) < 0):  # defensive: spec says idx is sorted
        order = np.argsort(idx, kind="stable")
        x, idx = x[order], idx[order]
    counts = np.bincount(idx, minlength=B)
    assert counts.sum() == x.shape[0]
    bounds = np.concatenate([[0], np.cumsum(counts)]).astype(np.int64)
    blk_rows = counts.reshape(NCORES * NBLK, SBLK).sum(1)
    tblk = int(np.ceil(blk_rows.max() / P))
    tblk = ((tblk + 3) // 4) * 4  # multiple of 4 -> NP % 1024 == 0
    NP = NBLK * tblk * P
    xs = np.zeros((NCORES, NP, DIN), f32)
    ixs = np.full((NCORES, NP), 1.0e9, f32)
    nchunks = NP // CH
    for c in range(NCORES):
        for blk in range(NBLK):
            s0 = c * SPC + blk * SBLK
            r0, r1 = int(bounds[s0]), int(bounds[s0 + SBLK])
            d0 = blk * tblk * P
            xs[c, d0 : d0 + (r1 - r0)] = x[r0:r1]
            ixs[c, d0 : d0 + (r1 - r0)] = (idx[r0:r1] - c * SPC).astype(f32)
    # pre-arrange idx so each partition's DMA read is contiguous:
    # ixs_arr[c, ch, p, n] = ixs[c, ch*CH + n*P + p]
    ixs_arr = np.ascontiguousarray(
        ixs.reshape(NCORES, nchunks, CH // P, P).transpose(0, 1, 3, 2)
    )
    return xs, ixs_arr, tblk, counts


def _build(tblk, phi_w1, phi_b1, phi_w2, phi_b2, rho_w1, rho_b1, rho_w2, rho_b2):
    import concourse.bacc as bacc
    import concourse.mybir as mybir
    import concourse.tile as tile

    BF = mybir.dt.bfloat16
    F32 = mybir.dt.float32
    Relu = mybir.ActivationFunctionType.Relu
    Copy = mybir.ActivationFunctionType.Copy

    has_b1 = bool(np.any(phi_b1 != 0))
    has_b2 = bool(np.any(phi_b2 != 0))
    has_rb1 = bool(np.any(rho_b1 != 0))
    has_rb2 = bool(np.any(rho_b2 != 0))

    # ---- packed constants (inlined into the NEFF) ----
    w1p = np.zeros((DIN, HPAD), f32)
    w1p[:, :DHID] = phi_w1
    if USE_FP8_MM1:
        w1k = np.ascontiguousarray(
            (w1p * FP8_SCALE).reshape(KC1, P, HPAD).transpose(1, 0, 2)
        ).astype(fp8)
    else:
        w1k = np.ascontiguousarray(w1p.reshape(KC1, P, HPAD).transpose(1, 0, 2)).astype(bf16)
    w2p = np.zeros((HPAD, DHID), f32)
    w2p[:DHID] = phi_w2
    if USE_FP8_MM1:
        w2p /= FP8_SCALE
    w2k = np.ascontiguousarray(w2p.reshape(2, P, DHID).transpose(1, 0, 2)).astype(bf16)
    rw1k = np.ascontiguousarray(rho_w1.reshape(2, 96, 6).transpose(1, 0, 2)).astype(f32)
    rw2k = np.ascontiguousarray(rho_w2).astype(f32)  # [6, 1]
    idn16 = np.eye(P, dtype=bf16)
    idn32 = np.eye(P, dtype=f32)
    jmat = np.ascontiguousarray(
        np.broadcast_to(
            (np.arange(NBLK)[:, None] * SBLK + np.arange(SBLK)[None, :]).astype(f32),
            (P, NBLK, SBLK),
        )
    )
    b1k = np.ascontiguousarray(
        np.concatenate([phi_b1, np.zeros(HPAD - DHID, f32)]).reshape(2, P).T
    ).astype(f32)  # [128, 2]
    rb1k = np.ascontiguousarray(rho_b1.reshape(6, 1)).astype(f32)
    rb2k = np.ascontiguousarray(rho_b2.reshape(1, 1)).astype(f32)
    onesk = np.ones((1, P), bf16)
    b2k = np.ascontiguousarray(phi_b2.reshape(1, DHID)).astype(bf16)

    NP = NBLK * tblk * P
    nchunks = NP // CH

    nc = bacc.Bacc(None, target_bir_lowering=False)
    x_in = nc.dram_tensor("x_shard", [NP, DIN], F32, kind="ExternalInput")
    ix_in = nc.dram_tensor("idxlf", [NP // CH, P, CH // P], F32, kind="ExternalInput")
    out_d = nc.dram_tensor("out_shard", [SPC], F32, kind="ExternalOutput")

    w1d = nc.inline_tensor(w1k, "w1k")
    w2d = nc.inline_tensor(w2k, "w2k")
    rw1d = nc.inline_tensor(rw1k, "rw1k")
    rw2d = nc.inline_tensor(rw2k, "rw2k")
    idn16d = nc.inline_tensor(idn16, "idn16")
    idn32d = nc.inline_tensor(idn32, "idn32")
    jmatd = nc.inline_tensor(jmat, "jmat")
    b1d = nc.inline_tensor(b1k, "b1k") if has_b1 else None
    rb1d = nc.inline_tensor(rb1k, "rb1k") if has_rb1 else None
    rb2d = nc.inline_tensor(rb2k, "rb2k") if has_rb2 else None
    onesd = nc.inline_tensor(onesk, "onesk") if has_b2 else None
    b2d = nc.inline_tensor(b2k, "b2k") if has_b2 else None

    with tile.TileContext(nc) as tc:
        with (
            tc.tile_pool(name="consts", bufs=1) as cpool,
            tc.tile_pool(name="xb", bufs=4) as xpool,
            tc.tile_pool(name="ixb", bufs=4) as ixpool,
            tc.tile_pool(name="xtb", bufs=3) as xtpool,
            tc.tile_pool(name="h1tb", bufs=2) as h1pool,
            tc.tile_pool(name="h2b", bufs=4) as h2pool,
            tc.tile_pool(name="selb", bufs=6) as selpool,
            tc.tile_pool(name="rho", bufs=1) as rhopool,
            tc.tile_pool(name="pxt", bufs=2, space="PSUM") as pxt,
            tc.tile_pool(name="ph1", bufs=2, space="PSUM") as ph1,
            tc.tile_pool(name="ph2", bufs=2, space="PSUM") as ph2,
            tc.tile_pool(name="pseg", bufs=2, space="PSUM") as pseg,
        ):
            # ---- load constants into SBUF ----
            w1s = cpool.tile_from(w1d[:])
            w2s = cpool.tile_from(w2d[:])
            rw1s = cpool.tile_from(rw1d[:])
            rw2s = cpool.tile_from(rw2d[:])
            idn16s = cpool.tile_from(idn16d[:])
            idn32s = cpool.tile_from(idn32d[:])
            js = cpool.tile_from(jmatd[:])
            b1s = cpool.tile_from(b1d[:]) if has_b1 else None
            rb1s = cpool.tile_from(rb1d[:]) if has_rb1 else None
            rb2s = cpool.tile_from(rb2d[:]) if has_rb2 else None
            oness = cpool.tile_from(onesd[:]) if has_b2 else None
            b2s = cpool.tile_from(b2d[:]) if has_b2 else None

            pseg_tiles = {}
            prev = None  # (h1tb, mblock_global_idx, ixb) pending mm2/seg tail

            def emit_tail(state, pair):
                """mm2 + segment-reduce for 2 sub-tiles of an earlier mblock."""
                h1tb_p, mg_p, ixb_p = state
                ph2t = ph2.tile([P, 2, DHID], F32, tag="h2", name=f"ph2_{mg_p}_{pair}")
                for j in range(2):
                    i = pair * 2 + j
                    for mc in range(2):
                        nc.tensor.matmul(
                            out=ph2t[:, j, :],
                            lhsT=h1tb_p[:, mc, i * P : (i + 1) * P],
                            rhs=w2s[:, mc, :],
                            start=(mc == 0),
                            stop=(mc == 1 and not has_b2),
                        )
                    if has_b2:
                        nc.tensor.matmul(
                            out=ph2t[:, j, :], lhsT=oness[:], rhs=b2s[:],
                            start=False, stop=True,
                        )
                h2bt = h2pool.tile([P, 2, DHID], BF, tag="h2b", name=f"h2b_{mg_p}_{pair}")
                if pair == 0:
                    nc.vector.tensor_copy(out=h2bt[:], in_=ph2t[:])
                else:
                    nc.scalar.copy(out=h2bt[:], in_=ph2t[:])
                for j in range(2):
                    i = pair * 2 + j
                    t = mg_p * 4 + i  # global sub-tile idx
                    blk = t // tblk
                    if t % tblk == 0:
                        pseg_tiles[blk] = pseg.tile([P, DHID], F32, tag="seg", name=f"pseg_{blk}")
                    selb = selpool.tile([P, P], BF, tag="selb", name=f"sel_{t}")
                    nloc = (mg_p % 2) * 4 + i
                    nc.vector.tensor_tensor(
                        out=selb[:],
                        in0=ixb_p[:, nloc : nloc + 1].to_broadcast([P, P]),
                        in1=js[:, blk, :],
                        op=mybir.AluOpType.is_equal,
                    )
                    nc.tensor.matmul(
                        out=pseg_tiles[blk][:],
                        lhsT=selb[:],
                        rhs=h2bt[:, j, :],
                        start=(t % tblk == 0),
                        stop=(t % tblk == tblk - 1),
                    )
                    if t % tblk == tblk - 1:
                        emit_rho(blk, pseg_tiles.pop(blk))

            def emit_rho(blk, pseg_t):
                # x_sum [128 segs, 192] f32 in psum -> out[blk*128:(blk+1)*128]
                xsb = rhopool.tile([P, DHID], F32, tag="xsb")
                nc.scalar.copy(out=xsb[:], in_=pseg_t[:])
                pxsT = pxt.tile([96, 2, P], F32, tag="xt")
                for m2 in range(2):
                    nc.tensor.transpose(
                        out=pxsT[:, m2, :],
                        in_=xsb[:, m2 * 96 : (m2 + 1) * 96],
                        identity=idn32s[:],
                    )
                xsTb = rhopool.tile([96, 2, P], F32, tag="xsTb")
                nc.vector.tensor_copy(out=xsTb[:], in_=pxsT[:])
                prt = pxt.tile([6, P], F32, tag="xt")
                for m2 in range(2):
                    nc.tensor.matmul(
                        out=prt[:],
                        lhsT=rw1s[:, m2, :],
                        rhs=xsTb[:, m2, :],
                        start=(m2 == 0),
                        stop=(m2 == 1),
                    )
                rtb = rhopool.tile([6, P], F32, tag="rtb")
                if has_rb1:
                    nc.scalar.activation(out=rtb[:], in_=prt[:], func=Relu, bias=rb1s[:])
                else:
                    nc.scalar.activation(out=rtb[:], in_=prt[:], func=Relu)
                pot = pxt.tile([1, P], F32, tag="xt")
                nc.tensor.matmul(out=pot[:], lhsT=rw2s[:], rhs=rtb[:], start=True, stop=True)
                ob = rhopool.tile([1, P], F32, tag="ob")
                if has_rb2:
                    nc.scalar.activation(out=ob[:], in_=pot[:], func=Copy, bias=rb2s[:])
                else:
                    nc.scalar.copy(out=ob[:], in_=pot[:])
                nc.sync.dma_start(out=out_d[blk * SBLK : (blk + 1) * SBLK], in_=ob[:])

            for ch in range(nchunks):
                r0 = ch * CH
                if ch == 0:
                    # split the first chunk into 4 pieces so PE starts ~7us sooner
                    xb0 = []
                    for q in range(8):
                        xq = xpool.tile([P, DIN], BF, tag=f"xb0_{q}", name=f"xb0_{q}", bufs=1)
                        nc.gpsimd.dma_start(
                            out=xq[:],
                            in_=x_in[q * P : (q + 1) * P, :],
                        )
                        xb0.append(xq)

                    def xb_at(n, kc):
                        return xb0[n][:, kc * P : (kc + 1) * P]

                    def xb_full(n):
                        return xb0[n][:]
                else:
                    xb = xpool.tile([P, CH // P, DIN], BF, tag="xb")
                    nc.gpsimd.dma_start(
                        out=xb[:],
                        in_=x_in[r0 : r0 + CH, :].rearrange("(n p) f -> p n f", p=P),
                    )

                    def xb_at(n, kc):
                        return xb[:, n, kc * P : (kc + 1) * P]

                    def xb_full(n):
                        return xb[:, n, :]

                ixb = ixpool.tile([P, CH // P], F32, tag="ixb")
                nc.sync.dma_start(out=ixb[:], in_=ix_in[ch])
                for half in range(2):  # 512-row mblocks
                    mg = ch * 2 + half
                    # --- transposes: x [rows, feat] -> xT [feat, rows],
                    # interleaved with the PREVIOUS mblock's mm2/seg tail so
                    # the short matmuls' weight loads hide under PE streams.
                    xtb = xtpool.tile([P, KC1, 512], mybir.dt.float8e4 if USE_FP8_MM1 else BF, tag="xtb")
                    for i in range(4):
                        n = half * 4 + i
                        pxtt = pxt.tile([P, KC1, P], BF, tag="xt", name=f"pxt_{half}_{i}")
                        for kc in range(KC1):
                            nc.tensor.transpose(
                                out=pxtt[:, kc, :],
                                in_=xb_at(n, kc),
                                identity=idn16s[:],
                            )
                        if i % 2 == 0:
                            nc.vector.tensor_copy(
                                out=xtb[:, :, i * P : (i + 1) * P], in_=pxtt[:]
                            )
                        else:
                            nc.scalar.copy(
                                out=xtb[:, :, i * P : (i + 1) * P], in_=pxtt[:]
                            )
                        if prev is not None and i % 2 == 1:
                            emit_tail(prev, i // 2)
                    # --- mm1: h1T [hid, 512] = W1.T @ xT ---
                    ph1s = [ph1.tile([P, 512], F32, tag="h1", name=f"ph1_{half}_{mc}") for mc in range(2)]
                    for mc in range(2):
                        for kc in range(KC1):
                            nc.tensor.matmul(
                                out=ph1s[mc][:],
                                lhsT=w1s[:, kc, mc * P : (mc + 1) * P],
                                rhs=xtb[:, kc, :],
                                start=(kc == 0),
                                stop=(kc == KC1 - 1),
                            )
                    h1tb = h1pool.tile([P, 2, 512], BF, tag="h1tb")
                    for mc in range(2):
                        if has_b1:
                            nc.scalar.activation(
                                out=h1tb[:, mc, :], in_=ph1s[mc][:], func=Relu,
                                bias=b1s[:, mc : mc + 1],
                            )
                        else:
                            nc.scalar.activation(
                                out=h1tb[:, mc, :], in_=ph1s[mc][:], func=Relu
                            )
                    prev = (h1tb, mg, ixb)
            # drain the last mblock's tail
            emit_tail(prev, 0)
            emit_tail(prev, 1)

    nc.compile()
    return nc


_CACHE = {}


def _get_nc(tblk, weights):
    key = tblk
    if key not in _CACHE:
        _CACHE[key] = _build(tblk, *weights)
    return _CACHE[key]


def _run(inputs, trace=False):
    from concourse.bass_utils import run_bass_kernel_spmd

    inp = {k: np.asarray(v) for k, v in inputs.items()}
    x = inp["x"].astype(f32, copy=False)
    idx = inp["idx"].astype(np.int32, copy=False)
    weights = tuple(
        inp[k].astype(f32, copy=False)
        for k in ("phi_w1", "phi_b1", "phi_w2", "phi_b2", "rho_w1", "rho_b1", "rho_w2", "rho_b2")
    )
    xs, ixs, tblk, counts = _prep(x, idx)
    nc = _get_nc(tblk, weights)
    in_maps = [{"x_shard": xs[c], "idxlf": ixs[c]} for c in range(NCORES)]
    res = run_bass_kernel_spmd(nc, in_maps, core_ids=list(range(NCORES)), trace=trace)
    out = np.concatenate([res.results[c]["out_shard"] for c in range(NCORES)])
    out = out.reshape(B, 1).astype(f32)
    # safety net: empty segments (never happens for the target distribution)
    if np.any(counts == 0):
        (phi_w1, phi_b1, phi_w2, phi_b2, rho_w1, rho_b1, rho_w2, rho_b2) = weights
        z = np.zeros((1, DHID), f32)
        r = np.maximum(z @ rho_w1 + rho_b1, 0.0)
        o0 = (r @ rho_w2 + rho_b2).astype(f32)
        out[counts == 0] = o0
    return out, res


def kernel(**inputs) -> np.ndarray:
    return _run(inputs, trace=False)[0]


if __name__ == "__main__":
    # quick self-test against numpy
    rng = np.random.default_rng(0)
    x = rng.standard_normal((N, DIN)).astype(f32)
    idx = np.sort(rng.integers(0, B, N).astype(np.int32))
    w1 = (rng.standard_normal((DIN, DHID)) / np.sqrt(DIN)).astype(f32)
    w2 = (rng.standard_normal((DHID, DHID)) / np.sqrt(DHID)).astype(f32)
    r1 = (rng.standard_normal((DHID, 6)) / np.sqrt(DHID)).astype(f32)
    r2 = (rng.standard_normal((6, 1)) / np.sqrt(6)).astype(f32)
    inputs = dict(
        x=x, idx=idx,
        phi_w1=w1, phi_b1=np.zeros(DHID, f32), phi_w2=w2, phi_b2=np.zeros(DHID, f32),
        rho_w1=r1, rho_b1=np.zeros(6, f32), rho_w2=r2, rho_b2=np.zeros(1, f32),
    )
    out = kernel(**inputs)
    h = np.maximum(x @ w1, 0.0) @ w2
    xsum = np.zeros((B, DHID), f32)
    np.add.at(xsum, idx, h)
    exp = np.maximum(xsum @ r1, 0.0) @ r2
    rel = np.linalg.norm(out - exp) / np.linalg.norm(exp)
    print("self-test rel err:", rel)



# revision 2
# speedup vs baseline: 2.0244x; 2.0244x over previous
"""Trainium2 Bass kernel for NeuronInvariantDeepSetLayer (segment_reduce).

kernel(**inputs) takes FULL unsharded inputs (as in reference.setup_inputs())
and returns the full [4096, 1] float32 output.

Strategy: data-parallel over 8 NeuronCores, 512 segments/core (idx is sorted,
so each core's rows are a contiguous slice of x). Rows are host-padded so each
128-segment block starts at a 128-row tile boundary -> identical SPMD
instruction stream on all cores.

Key algebraic fold: segment_sum commutes with the second (linear) phi layer:
    x_sum = segsum(relu(x@W1+b1) @ W2 + b2)
          = segsum(relu(x@W1+b1)) @ W2 + counts*b2
and W2 then folds into rho:  x_sum @ rho_w1 = segsum(h1r) @ (W2@rho_w1) + ...
So the device only computes mm1 + segment-reduce + a tiny per-block rho with
V = W2@rho_w1 [192,6]. mm2 never materializes.

Host prep: x is cast to bf16 AND pre-transposed per core to [128, 6, NP]
(feature-on-partition layout), halving HBM traffic and removing all PE
transposes of x. Device pipeline per 128-row tile:
  - 6 matmuls (lhsT = xT tile chunk, rhs = W1 chunk [128,192]) -> psum h1
  - ACT relu psum -> SBUF bf16 h1r [rows, 192]
  - DVE one-hot sel = is_equal(idx_local, iota) [rows, 128 segs]
  - 1 matmul pseg[blk] += sel.T @ h1r, PSUM-accumulated over ~tblk tiles
Per 128-seg block: tiny rho (transpose x_sum, x_sum@V, relu, @rho_w2) -> out.
"""

import sys

sys.path.insert(0, "/opt/trn_rl_repo")

import numpy as np
import ml_dtypes

N = 400000
B = 4096
DIN = 768
DHID = 192
NCORES = 8
SPC = B // NCORES  # segments per core = 512
SBLK = 128  # segments per seg-block (psum accumulator height)
NBLK = SPC // SBLK  # 4 seg-blocks per core
P = 128
KC1 = DIN // P  # 6 k-chunks for mm1
CH = 2048  # rows per steady-state x DMA chunk (16 tiles)

f32 = np.float32
bf16 = ml_dtypes.bfloat16


def _prep(x, idx):
    """Host-side sharding: per-core bf16 transposed x + local idx layout."""
    if np.any(np.diff(idx) < 0):  # defensive: spec says idx is sorted
        order = np.argsort(idx, kind="stable")
        x, idx = x[order], idx[order]
    counts = np.bincount(idx, minlength=B)
    assert counts.sum() == x.shape[0]
    bounds = np.concatenate([[0], np.cumsum(counts)]).astype(np.int64)
    blk_rows = counts.reshape(NCORES * NBLK, SBLK).sum(1)
    tblk = int(np.ceil(blk_rows.max() / P))
    tblk = ((tblk + 3) // 4) * 4  # multiple of 4 -> NP % 2048 == 0
    NP = NBLK * tblk * P
    ntiles = NP // P
    xs = np.zeros((NCORES, P, KC1, NP), bf16)  # xs[c, p, k, r] = x[r, k*128+p]
    ixs = np.full((NCORES, NP), 1.0e9, f32)
    for c in range(NCORES):
        for blk in range(NBLK):
            s0 = c * SPC + blk * SBLK
            r0, r1 = int(bounds[s0]), int(bounds[s0 + SBLK])
            nr = r1 - r0
            d0 = blk * tblk * P
            xs[c, :, :, d0 : d0 + nr] = (
                x[r0:r1].T.reshape(KC1, P, nr).transpose(1, 0, 2)
            )
            ixs[c, d0 : d0 + nr] = (idx[r0:r1] - c * SPC).astype(f32)
    # ix layout: [128, ntiles], col t = local idx of rows t*128 .. t*128+127
    ixarr = np.ascontiguousarray(ixs.reshape(NCORES, ntiles, P).transpose(0, 2, 1))
    return xs, ixarr, tblk, counts


def _build(tblk, phi_w1, phi_b1, phi_w2, phi_b2, rho_w1, rho_b1, rho_w2, rho_b2):
    import concourse.bacc as bacc
    import concourse.mybir as mybir
    import concourse.tile as tile

    BF = mybir.dt.bfloat16
    F32 = mybir.dt.float32
    Relu = mybir.ActivationFunctionType.Relu
    Copy = mybir.ActivationFunctionType.Copy

    has_b1 = bool(np.any(phi_b1 != 0))
    has_b2 = bool(np.any(phi_b2 != 0))
    has_rb1 = bool(np.any(rho_b1 != 0))
    has_rb2 = bool(np.any(rho_b2 != 0))

    # ---- packed constants (inlined into the NEFF) ----
    # W1 as mm1 rhs: [128 (feat chunk part), 6, 192]
    w1k = np.ascontiguousarray(
        phi_w1.reshape(KC1, P, DHID).transpose(1, 0, 2)
    ).astype(bf16)
    # V = W2 @ rho_w1 folds mm2 into rho. lhsT chunks: [96, 2, 6]
    V = (phi_w2 @ rho_w1).astype(f32)
    rvk = np.ascontiguousarray(V.reshape(2, 96, 6).transpose(1, 0, 2)).astype(f32)
    rw2k = np.ascontiguousarray(rho_w2).astype(f32)  # [6, 1]
    idn32 = np.eye(P, dtype=f32)
    jmat = np.ascontiguousarray(
        np.broadcast_to(
            (np.arange(NBLK)[:, None] * SBLK + np.arange(SBLK)[None, :]).astype(f32),
            (P, NBLK, SBLK),
        )
    )
    rb1k = np.ascontiguousarray(rho_b1.reshape(6, 1)).astype(f32)
    rb2k = np.ascontiguousarray(rho_b2.reshape(1, 1)).astype(f32)
    ones1 = np.ones((1, P), bf16)
    b1row = np.ascontiguousarray(phi_b1.reshape(1, DHID)).astype(bf16)
    c2k = np.ascontiguousarray((phi_b2 @ rho_w1).reshape(1, 6)).astype(f32)

    NP = NBLK * tblk * P
    ntiles = NP // P
    nch = NP // CH
    TPC = CH // P  # tiles per chunk = 16

    nc = bacc.Bacc(None, target_bir_lowering=False)
    xt_in = nc.dram_tensor("xt", [P, KC1, NP], BF, kind="ExternalInput")
    ix_in = nc.dram_tensor("ixl", [P, ntiles], F32, kind="ExternalInput")
    cnt_in = (
        nc.dram_tensor("cnts", [1, SPC], F32, kind="ExternalInput") if has_b2 else None
    )
    out_d = nc.dram_tensor("out_shard", [SPC], F32, kind="ExternalOutput")

    w1d = nc.inline_tensor(w1k, "w1k")
    rvd = nc.inline_tensor(rvk, "rvk")
    rw2d = nc.inline_tensor(rw2k, "rw2k")
    idn32d = nc.inline_tensor(idn32, "idn32")
    jmatd = nc.inline_tensor(jmat, "jmat")
    rb1d = nc.inline_tensor(rb1k, "rb1k") if has_rb1 else None
    rb2d = nc.inline_tensor(rb2k, "rb2k") if has_rb2 else None
    ones1d = nc.inline_tensor(ones1, "ones1") if has_b1 else None
    b1rd = nc.inline_tensor(b1row, "b1row") if has_b1 else None
    c2d = nc.inline_tensor(c2k, "c2k") if has_b2 else None

    with tile.TileContext(nc) as tc:
        with (
            tc.tile_pool(name="consts", bufs=1) as cpool,
            tc.tile_pool(name="xb", bufs=4) as xpool,
            tc.tile_pool(name="h1b", bufs=6) as h1pool,
            tc.tile_pool(name="selb", bufs=6) as selpool,
            tc.tile_pool(name="rho", bufs=1) as rhopool,
            tc.tile_pool(name="ph1", bufs=4, space="PSUM") as ph1,
            tc.tile_pool(name="pseg", bufs=2, space="PSUM") as pseg,
            tc.tile_pool(name="pxt", bufs=2, space="PSUM") as pxt,
        ):
            # ---- constants into SBUF ----
            w1s = cpool.tile_from(w1d[:])
            rvs = cpool.tile_from(rvd[:])
            rw2s = cpool.tile_from(rw2d[:])
            idn32s = cpool.tile_from(idn32d[:])
            js = cpool.tile_from(jmatd[:])
            rb1s = cpool.tile_from(rb1d[:]) if has_rb1 else None
            rb2s = cpool.tile_from(rb2d[:]) if has_rb2 else None
            ones1s = cpool.tile_from(ones1d[:]) if has_b1 else None
            b1rs = cpool.tile_from(b1rd[:]) if has_b1 else None
            c2s = cpool.tile_from(c2d[:]) if has_b2 else None
            ixs = cpool.tile([P, ntiles], F32, tag="ixs")
            nc.sync.dma_start(out=ixs[:], in_=ix_in[:])
            cnts = None
            if has_b2:
                cnts = cpool.tile([1, SPC], F32, tag="cnts")
                nc.sync.dma_start(out=cnts[:], in_=cnt_in[:])

            pseg_tiles = {}
            prev = None  # (t, blk, selt, h1t) pending segment-reduce matmul

            def emit_rho(blk, pt):
                # x_sum [128 segs, 192] f32 psum -> out[blk*128:(blk+1)*128]
                xsb = rhopool.tile([P, DHID], F32, tag="xsb")
                nc.scalar.copy(out=xsb[:], in_=pt[:])
                pxsT = pxt.tile([96, 2, P], F32, tag="xt")
                for m2 in range(2):
                    nc.tensor.transpose(
                        out=pxsT[:, m2, :],
                        in_=xsb[:, m2 * 96 : (m2 + 1) * 96],
                        identity=idn32s[:],
                    )
                xsTb = rhopool.tile([96, 2, P], F32, tag="xsTb")
                nc.vector.tensor_copy(out=xsTb[:], in_=pxsT[:])
                prt = pxt.tile([6, P], F32, tag="xt")
                for m2 in range(2):
                    nc.tensor.matmul(
                        out=prt[:],
                        lhsT=rvs[:, m2, :],
                        rhs=xsTb[:, m2, :],
                        start=(m2 == 0),
                        stop=(m2 == 1 and not has_b2),
                    )
                if has_b2:
                    # += b2@rho_w1 (outer) counts  (K=1 matmul)
                    nc.tensor.matmul(
                        out=prt[:],
                        lhsT=c2s[:],
                        rhs=cnts[:, blk * SBLK : (blk + 1) * SBLK],
                        start=False,
                        stop=True,
                    )
                rtb = rhopool.tile([6, P], F32, tag="rtb")
                if has_rb1:
                    nc.scalar.activation(out=rtb[:], in_=prt[:], func=Relu, bias=rb1s[:])
                else:
                    nc.scalar.activation(out=rtb[:], in_=prt[:], func=Relu)
                pot = pxt.tile([1, P], F32, tag="xt")
                nc.tensor.matmul(out=pot[:], lhsT=rw2s[:], rhs=rtb[:], start=True, stop=True)
                ob = rhopool.tile([1, P], F32, tag="ob")
                if has_rb2:
                    nc.scalar.activation(out=ob[:], in_=pot[:], func=Copy, bias=rb2s[:])
                else:
                    nc.scalar.copy(out=ob[:], in_=pot[:])
                nc.sync.dma_start(out=out_d[blk * SBLK : (blk + 1) * SBLK], in_=ob[:])

            def emit_seg(st):
                t, blk, selt, h1t = st
                first = t % tblk == 0
                last = t % tblk == tblk - 1
                if first:
                    pseg_tiles[blk] = pseg.tile(
                        [P, DHID], F32, tag="seg", name=f"pseg_{blk}"
                    )
                nc.tensor.matmul(
                    out=pseg_tiles[blk][:], lhsT=selt[:], rhs=h1t[:],
                    start=first, stop=last,
                )
                if last:
                    emit_rho(blk, pseg_tiles.pop(blk))

            for ch in range(nch):
                r0 = ch * CH
                if ch == 0:
                    # split the first chunk so PE starts ~6us sooner
                    xparts = []
                    for q in range(4):
                        xq = xpool.tile(
                            [P, KC1, 512], BF, tag=f"x0_{q}", name=f"x0_{q}", bufs=1
                        )
                        nc.gpsimd.dma_start(
                            out=xq[:], in_=xt_in[:, :, q * 512 : (q + 1) * 512]
                        )
                        xparts.append(xq)

                    def lhs_at(s, k):
                        return xparts[s // 4][:, k, (s % 4) * P : (s % 4 + 1) * P]

                else:
                    xtb = xpool.tile([P, KC1, CH], BF, tag="xtb")
                    nc.gpsimd.dma_start(out=xtb[:], in_=xt_in[:, :, r0 : r0 + CH])

                    def lhs_at(s, k, _x=xtb):
                        return _x[:, k, s * P : (s + 1) * P]

                for s in range(TPC):
                    t = ch * TPC + s
                    blk = t // tblk
                    ph1t = ph1.tile([P, DHID], F32, tag="h1", name=f"ph1_{t}")
                    for k in range(KC1):
                        nc.tensor.matmul(
                            out=ph1t[:],
                            lhsT=lhs_at(s, k),
                            rhs=w1s[:, k, :],
                            start=(k == 0),
                            stop=(k == KC1 - 1 and not has_b1),
                        )
                    if has_b1:
                        nc.tensor.matmul(
                            out=ph1t[:], lhsT=ones1s[:], rhs=b1rs[:],
                            start=False, stop=True,
                        )
                    h1t = h1pool.tile([P, DHID], BF, tag="h1b", name=f"h1b_{t}")
                    nc.scalar.activation(out=h1t[:], in_=ph1t[:], func=Relu)
                    selt = selpool.tile([P, P], BF, tag="sel", name=f"sel_{t}")
                    nc.vector.tensor_tensor(
                        out=selt[:],
                        in0=ixs[:, t : t + 1].to_broadcast([P, P]),
                        in1=js[:, blk, :],
                        op=mybir.AluOpType.is_equal,
                    )
                    if prev is not None:
                        emit_seg(prev)
                    prev = (t, blk, selt, h1t)
            emit_seg(prev)

    nc.compile()
    return nc


_CACHE = {}


def _get_nc(tblk, weights):
    key = tblk
    if key not in _CACHE:
        _CACHE[key] = _build(tblk, *weights)
    return _CACHE[key]


def _run(inputs, trace=False):
    from concourse.bass_utils import run_bass_kernel_spmd

    inp = {k: np.asarray(v) for k, v in inputs.items()}
    x = inp["x"].astype(f32, copy=False)
    idx = inp["idx"].astype(np.int32, copy=False)
    weights = tuple(
        inp[k].astype(f32, copy=False)
        for k in ("phi_w1", "phi_b1", "phi_w2", "phi_b2", "rho_w1", "rho_b1", "rho_w2", "rho_b2")
    )
    xs, ixarr, tblk, counts = _prep(x, idx)
    nc = _get_nc(tblk, weights)
    has_b2 = bool(np.any(weights[3] != 0))
    in_maps = []
    for c in range(NCORES):
        m = {"xt": xs[c], "ixl": ixarr[c]}
        if has_b2:
            m["cnts"] = np.ascontiguousarray(
                counts.reshape(NCORES, SPC)[c].reshape(1, SPC)
            ).astype(f32)
        in_maps.append(m)
    res = run_bass_kernel_spmd(nc, in_maps, core_ids=list(range(NCORES)), trace=trace)
    out = np.concatenate([res.results[c]["out_shard"] for c in range(NCORES)])
    out = out.reshape(B, 1).astype(f32)
    return out, res


def kernel(**inputs) -> np.ndarray:
    return _run(inputs, trace=False)[0]


if __name__ == "__main__":
    # quick self-test against numpy
    rng = np.random.default_rng(0)
    x = rng.standard_normal((N, DIN)).astype(f32)
    idx = np.sort(rng.integers(0, B, N).astype(np.int32))
    w1 = (rng.standard_normal((DIN, DHID)) / np.sqrt(DIN)).astype(f32)
    w2 = (rng.standard_normal((DHID, DHID)) / np.sqrt(DHID)).astype(f32)
    r1 = (rng.standard_normal((DHID, 6)) / np.sqrt(DHID)).astype(f32)
    r2 = (rng.standard_normal((6, 1)) / np.sqrt(6)).astype(f32)
    inputs = dict(
        x=x, idx=idx,
        phi_w1=w1, phi_b1=np.zeros(DHID, f32), phi_w2=w2, phi_b2=np.zeros(DHID, f32),
        rho_w1=r1, rho_b1=np.zeros(6, f32), rho_w2=r2, rho_b2=np.zeros(1, f32),
    )
    out = kernel(**inputs)
    h = np.maximum(x @ w1, 0.0) @ w2
    xsum = np.zeros((B, DHID), f32)
    np.add.at(xsum, idx, h)
    exp = np.maximum(xsum @ r1, 0.0) @ r2
    rel = np.linalg.norm(out - exp) / np.linalg.norm(exp)
    print("self-test rel err:", rel)


# revision 8
# speedup vs baseline: 2.0440x; 1.0097x over previous
"""Trainium2 Bass kernel for NeuronInvariantDeepSetLayer (segment_reduce).

kernel(**inputs) takes FULL unsharded inputs (as in reference.setup_inputs())
and returns the full [4096, 1] float32 output.

Strategy: data-parallel over 8 NeuronCores, 512 segments/core (idx is sorted,
so each core's rows are a contiguous slice of x). Rows are host-padded so each
128-segment block starts at a 128-row tile boundary -> identical SPMD
instruction stream on all cores.

Key algebraic fold: segment_sum commutes with the second (linear) phi layer:
    x_sum = segsum(relu(x@W1+b1) @ W2 + b2)
          = segsum(relu(x@W1+b1)) @ W2 + counts*b2
and W2 then folds into rho:  x_sum @ rho_w1 = segsum(h1r) @ (W2@rho_w1) + ...
So the device only computes mm1 + segment-reduce + a tiny per-block rho with
V = W2@rho_w1 [192,6]. mm2 never materializes.

Host prep: x is cast to bf16 AND pre-transposed per core to [128, 6, NP]
(feature-on-partition layout), halving HBM traffic and removing all PE
transposes of x. Device pipeline per 128-row tile:
  - 6 matmuls (lhsT = xT tile chunk, rhs = W1 chunk [128,192]) -> psum h1
  - ACT relu psum -> SBUF bf16 h1r [rows, 192]
  - DVE one-hot sel = is_equal(idx_local, iota) [rows, 128 segs]
  - 1 matmul pseg[blk] += sel.T @ h1r, PSUM-accumulated over ~tblk tiles
Per 128-seg block: tiny rho (transpose x_sum, x_sum@V, relu, @rho_w2) -> out.
"""

import sys

sys.path.insert(0, "/opt/trn_rl_repo")

import numpy as np
import ml_dtypes

N = 400000
B = 4096
DIN = 768
DHID = 192
NCORES = 8
SPC = B // NCORES  # segments per core = 512
SBLK = 128  # segments per seg-block (psum accumulator height)
NBLK = SPC // SBLK  # 4 seg-blocks per core
P = 128
KC1 = DIN // P  # 6 k-chunks for mm1
CH = 2048  # rows per steady-state x DMA chunk (16 tiles)

f32 = np.float32
bf16 = ml_dtypes.bfloat16


def _prep(x, idx):
    """Host-side sharding: per-core bf16 transposed x + local idx layout."""
    if np.any(np.diff(idx) < 0):  # defensive: spec says idx is sorted
        order = np.argsort(idx, kind="stable")
        x, idx = x[order], idx[order]
    counts = np.bincount(idx, minlength=B)
    assert counts.sum() == x.shape[0]
    bounds = np.concatenate([[0], np.cumsum(counts)]).astype(np.int64)
    blk_rows = counts.reshape(NCORES * NBLK, SBLK).sum(1)
    tblk = int(np.ceil(blk_rows.max() / P))
    tblk = ((tblk + 3) // 4) * 4  # multiple of 4 -> NP % 2048 == 0
    NP = NBLK * tblk * P
    ntiles = NP // P
    xs = np.zeros((NCORES, P, KC1, NP), bf16)  # xs[c, p, k, r] = x[r, k*128+p]
    ixs = np.full((NCORES, NP), 1.0e9, f32)
    for c in range(NCORES):
        for blk in range(NBLK):
            s0 = c * SPC + blk * SBLK
            r0, r1 = int(bounds[s0]), int(bounds[s0 + SBLK])
            nr = r1 - r0
            d0 = blk * tblk * P
            xs[c, :, :, d0 : d0 + nr] = (
                x[r0:r1].T.reshape(KC1, P, nr).transpose(1, 0, 2)
            )
            ixs[c, d0 : d0 + nr] = (idx[r0:r1] - c * SPC).astype(f32)
    # ix layout: [128, ntiles], col t = local idx of rows t*128 .. t*128+127
    ixarr = np.ascontiguousarray(ixs.reshape(NCORES, ntiles, P).transpose(0, 2, 1))
    return xs, ixarr, tblk, counts


def _build(tblk, phi_w1, phi_b1, phi_w2, phi_b2, rho_w1, rho_b1, rho_w2, rho_b2):
    import concourse.bacc as bacc
    import concourse.mybir as mybir
    import concourse.tile as tile

    BF = mybir.dt.bfloat16
    F32 = mybir.dt.float32
    Relu = mybir.ActivationFunctionType.Relu
    Copy = mybir.ActivationFunctionType.Copy

    has_b1 = bool(np.any(phi_b1 != 0))
    has_b2 = bool(np.any(phi_b2 != 0))
    has_rb1 = bool(np.any(rho_b1 != 0))
    has_rb2 = bool(np.any(rho_b2 != 0))

    # ---- packed constants (inlined into the NEFF) ----
    # W1 as mm1 rhs: [128 (feat chunk part), 6, 192]
    w1k = np.ascontiguousarray(
        phi_w1.reshape(KC1, P, DHID).transpose(1, 0, 2)
    ).astype(bf16)
    # V = W2 @ rho_w1 folds mm2 into rho. lhsT chunks: [96, 2, 6]
    V = (phi_w2 @ rho_w1).astype(f32)
    rvk = np.ascontiguousarray(V.reshape(2, 96, 6).transpose(1, 0, 2)).astype(f32)
    rw2k = np.ascontiguousarray(rho_w2).astype(f32)  # [6, 1]
    idn32 = np.eye(P, dtype=f32)
    jmat = np.ascontiguousarray(
        np.broadcast_to(
            (np.arange(NBLK)[:, None] * SBLK + np.arange(SBLK)[None, :]).astype(f32),
            (P, NBLK, SBLK),
        )
    )
    rb1k = np.ascontiguousarray(rho_b1.reshape(6, 1)).astype(f32)
    rb2k = np.ascontiguousarray(rho_b2.reshape(1, 1)).astype(f32)
    ones1 = np.ones((1, P), bf16)
    b1row = np.ascontiguousarray(phi_b1.reshape(1, DHID)).astype(bf16)
    c2k = np.ascontiguousarray((phi_b2 @ rho_w1).reshape(1, 6)).astype(f32)

    NP = NBLK * tblk * P
    ntiles = NP // P
    nch = NP // CH
    TPC = CH // P  # tiles per chunk = 16
    # first chunk is split so PE starts as soon as possible (tile boundaries)
    CH0_SPLIT = (2, 2, 4, 8)

    nc = bacc.Bacc(None, target_bir_lowering=False)
    xt_in = nc.dram_tensor("xt", [P, KC1, NP], BF, kind="ExternalInput")
    ix_in = nc.dram_tensor("ixl", [P, ntiles], F32, kind="ExternalInput")
    cnt_in = (
        nc.dram_tensor("cnts", [1, SPC], F32, kind="ExternalInput") if has_b2 else None
    )
    out_d = nc.dram_tensor("out_shard", [SPC], F32, kind="ExternalOutput")

    w1d = nc.inline_tensor(w1k, "w1k")
    rvd = nc.inline_tensor(rvk, "rvk")
    rw2d = nc.inline_tensor(rw2k, "rw2k")
    idn32d = nc.inline_tensor(idn32, "idn32")
    jmatd = nc.inline_tensor(jmat, "jmat")
    rb1d = nc.inline_tensor(rb1k, "rb1k") if has_rb1 else None
    rb2d = nc.inline_tensor(rb2k, "rb2k") if has_rb2 else None
    ones1d = nc.inline_tensor(ones1, "ones1") if has_b1 else None
    b1rd = nc.inline_tensor(b1row, "b1row") if has_b1 else None
    c2d = nc.inline_tensor(c2k, "c2k") if has_b2 else None

    with tile.TileContext(nc) as tc:
        with (
            tc.tile_pool(name="consts", bufs=1) as cpool,
            tc.tile_pool(name="xb", bufs=4) as xpool,
            tc.tile_pool(name="ixb", bufs=4) as ixpool,
            tc.tile_pool(name="h1b", bufs=6) as h1pool,
            tc.tile_pool(name="selb", bufs=6) as selpool,
            tc.tile_pool(name="rho", bufs=1) as rhopool,
            tc.tile_pool(name="ph1", bufs=4, space="PSUM") as ph1,
            tc.tile_pool(name="pseg", bufs=2, space="PSUM") as pseg,
            tc.tile_pool(name="pxt", bufs=2, space="PSUM") as pxt,
        ):
            # ---- constants needed in the first microseconds ----
            w1s = cpool.tile_from(w1d[:])
            js = cpool.tile_from(jmatd[:])
            ones1s = cpool.tile_from(ones1d[:]) if has_b1 else None
            b1rs = cpool.tile_from(b1rd[:]) if has_b1 else None

            # rho-only constants: loaded lazily (first needed ~70us in) so the
            # startup DMA window is reserved for x / idx data.
            _rc = {}

            def rho_consts():
                if not _rc:
                    _rc["rvs"] = cpool.tile_from(rvd[:], name="rvs")
                    _rc["rw2s"] = cpool.tile_from(rw2d[:], name="rw2s")
                    _rc["idn32s"] = cpool.tile_from(idn32d[:], name="idn32s")
                    _rc["rb1s"] = cpool.tile_from(rb1d[:], name="rb1s") if has_rb1 else None
                    _rc["rb2s"] = cpool.tile_from(rb2d[:], name="rb2s") if has_rb2 else None
                    _rc["c2s"] = cpool.tile_from(c2d[:], name="c2s") if has_b2 else None
                    if has_b2:
                        cn = cpool.tile([1, SPC], F32, tag="cnts")
                        nc.sync.dma_start(out=cn[:], in_=cnt_in[:])
                        _rc["cnts"] = cn
                return _rc

            pseg_tiles = {}
            prev = None  # (t, blk, selt, h1t) pending segment-reduce matmul
            pending_rho = []  # [(blk, pseg_tile)] deferred one tile

            def emit_rho(blk, pt):
                rc = rho_consts()
                rvs, rw2s, idn32s = rc["rvs"], rc["rw2s"], rc["idn32s"]
                rb1s, rb2s, c2s = rc["rb1s"], rc["rb2s"], rc["c2s"]
                cnts = rc.get("cnts")
                # x_sum [128 segs, 192] f32 psum -> out[blk*128:(blk+1)*128]
                xsb = rhopool.tile([P, DHID], F32, tag="xsb")
                nc.scalar.copy(out=xsb[:], in_=pt[:])
                pxsT = pxt.tile([96, 2, P], F32, tag="xt")
                for m2 in range(2):
                    nc.tensor.transpose(
                        out=pxsT[:, m2, :],
                        in_=xsb[:, m2 * 96 : (m2 + 1) * 96],
                        identity=idn32s[:],
                    )
                xsTb = rhopool.tile([96, 2, P], F32, tag="xsTb")
                nc.vector.tensor_copy(out=xsTb[:], in_=pxsT[:])
                prt = pxt.tile([6, P], F32, tag="xt")
                for m2 in range(2):
                    nc.tensor.matmul(
                        out=prt[:],
                        lhsT=rvs[:, m2, :],
                        rhs=xsTb[:, m2, :],
                        start=(m2 == 0),
                        stop=(m2 == 1 and not has_b2),
                    )
                if has_b2:
                    # += b2@rho_w1 (outer) counts  (K=1 matmul)
                    nc.tensor.matmul(
                        out=prt[:],
                        lhsT=c2s[:],
                        rhs=cnts[:, blk * SBLK : (blk + 1) * SBLK],
                        start=False,
                        stop=True,
                    )
                rtb = rhopool.tile([6, P], F32, tag="rtb")
                if has_rb1:
                    nc.scalar.activation(out=rtb[:], in_=prt[:], func=Relu, bias=rb1s[:])
                else:
                    nc.scalar.activation(out=rtb[:], in_=prt[:], func=Relu)
                pot = pxt.tile([1, P], F32, tag="xt")
                nc.tensor.matmul(out=pot[:], lhsT=rw2s[:], rhs=rtb[:], start=True, stop=True)
                ob = rhopool.tile([1, P], F32, tag="ob")
                if has_rb2:
                    nc.scalar.activation(out=ob[:], in_=pot[:], func=Copy, bias=rb2s[:])
                else:
                    nc.scalar.copy(out=ob[:], in_=pot[:])
                nc.sync.dma_start(out=out_d[blk * SBLK : (blk + 1) * SBLK], in_=ob[:])

            def emit_seg(st):
                t, blk, selt, h1t = st
                first = t % tblk == 0
                last = t % tblk == tblk - 1
                if first:
                    pseg_tiles[blk] = pseg.tile(
                        [P, DHID], F32, tag="seg", name=f"pseg_{blk}"
                    )
                nc.tensor.matmul(
                    out=pseg_tiles[blk][:], lhsT=selt[:], rhs=h1t[:],
                    start=first, stop=last,
                )
                if last:
                    # defer rho by one tile: its PE ops then trail the next
                    # tile's mm1 stream instead of stalling the PE queue while
                    # the ACT copy of pseg drains.
                    pending_rho.append((blk, pseg_tiles.pop(blk)))

            for ch in range(nch):
                r0 = ch * CH
                ixc = ixpool.tile([P, TPC], F32, tag="ixb", name=f"ix_{ch}")
                nc.sync.dma_start(
                    out=ixc[:], in_=ix_in[:, ch * TPC : (ch + 1) * TPC]
                )
                if ch == 0:
                    # split the first chunk so PE starts as soon as possible
                    xparts = []  # (first_tile, ntile, tile)
                    tq = 0
                    for q, nt in enumerate(CH0_SPLIT):
                        xq = xpool.tile(
                            [P, KC1, nt * P], BF, tag=f"x0_{q}", name=f"x0_{q}", bufs=1
                        )
                        nc.gpsimd.dma_start(
                            out=xq[:], in_=xt_in[:, :, tq * P : (tq + nt) * P]
                        )
                        xparts.append((tq, nt, xq))
                        tq += nt

                    def lhs_at(s, k):
                        for q0, nt, xq in xparts:
                            if s < q0 + nt:
                                return xq[:, k, (s - q0) * P : (s - q0 + 1) * P]
                        raise AssertionError

                else:
                    xtb = xpool.tile([P, KC1, CH], BF, tag="xtb")
                    nc.gpsimd.dma_start(out=xtb[:], in_=xt_in[:, :, r0 : r0 + CH])

                    def lhs_at(s, k, _x=xtb):
                        return _x[:, k, s * P : (s + 1) * P]

                for s in range(TPC):
                    t = ch * TPC + s
                    blk = t // tblk
                    ph1t = ph1.tile([P, DHID], F32, tag="h1", name=f"ph1_{t}")
                    for k in range(KC1):
                        nc.tensor.matmul(
                            out=ph1t[:],
                            lhsT=lhs_at(s, k),
                            rhs=w1s[:, k, :],
                            start=(k == 0),
                            stop=(k == KC1 - 1 and not has_b1),
                        )
                    if has_b1:
                        nc.tensor.matmul(
                            out=ph1t[:], lhsT=ones1s[:], rhs=b1rs[:],
                            start=False, stop=True,
                        )
                    h1t = h1pool.tile([P, DHID], BF, tag="h1b", name=f"h1b_{t}")
                    nc.scalar.activation(out=h1t[:], in_=ph1t[:], func=Relu)
                    selt = selpool.tile([P, P], BF, tag="sel", name=f"sel_{t}")
                    nc.vector.tensor_tensor(
                        out=selt[:],
                        in0=ixc[:, s : s + 1].to_broadcast([P, P]),
                        in1=js[:, blk, :],
                        op=mybir.AluOpType.is_equal,
                    )
                    while pending_rho:
                        emit_rho(*pending_rho.pop(0))
                    if prev is not None:
                        emit_seg(prev)
                    prev = (t, blk, selt, h1t)
            emit_seg(prev)
            while pending_rho:
                emit_rho(*pending_rho.pop(0))

    nc.compile()
    return nc


_CACHE = {}


def _get_nc(tblk, weights):
    key = tblk
    if key not in _CACHE:
        _CACHE[key] = _build(tblk, *weights)
    return _CACHE[key]


def _run(inputs, trace=False):
    from concourse.bass_utils import run_bass_kernel_spmd

    inp = {k: np.asarray(v) for k, v in inputs.items()}
    x = inp["x"].astype(f32, copy=False)
    idx = inp["idx"].astype(np.int32, copy=False)
    weights = tuple(
        inp[k].astype(f32, copy=False)
        for k in ("phi_w1", "phi_b1", "phi_w2", "phi_b2", "rho_w1", "rho_b1", "rho_w2", "rho_b2")
    )
    xs, ixarr, tblk, counts = _prep(x, idx)
    nc = _get_nc(tblk, weights)
    has_b2 = bool(np.any(weights[3] != 0))
    in_maps = []
    for c in range(NCORES):
        m = {"xt": xs[c], "ixl": ixarr[c]}
        if has_b2:
            m["cnts"] = np.ascontiguousarray(
                counts.reshape(NCORES, SPC)[c].reshape(1, SPC)
            ).astype(f32)
        in_maps.append(m)
    res = run_bass_kernel_spmd(nc, in_maps, core_ids=list(range(NCORES)), trace=trace)
    out = np.concatenate([res.results[c]["out_shard"] for c in range(NCORES)])
    out = out.reshape(B, 1).astype(f32)
    return out, res


def kernel(**inputs) -> np.ndarray:
    return _run(inputs, trace=False)[0]


if __name__ == "__main__":
    # quick self-test against numpy
    rng = np.random.default_rng(0)
    x = rng.standard_normal((N, DIN)).astype(f32)
    idx = np.sort(rng.integers(0, B, N).astype(np.int32))
    w1 = (rng.standard_normal((DIN, DHID)) / np.sqrt(DIN)).astype(f32)
    w2 = (rng.standard_normal((DHID, DHID)) / np.sqrt(DHID)).astype(f32)
    r1 = (rng.standard_normal((DHID, 6)) / np.sqrt(DHID)).astype(f32)
    r2 = (rng.standard_normal((6, 1)) / np.sqrt(6)).astype(f32)
    inputs = dict(
        x=x, idx=idx,
        phi_w1=w1, phi_b1=np.zeros(DHID, f32), phi_w2=w2, phi_b2=np.zeros(DHID, f32),
        rho_w1=r1, rho_b1=np.zeros(6, f32), rho_w2=r2, rho_b2=np.zeros(1, f32),
    )
    out = kernel(**inputs)
    h = np.maximum(x @ w1, 0.0) @ w2
    xsum = np.zeros((B, DHID), f32)
    np.add.at(xsum, idx, h)
    exp = np.maximum(xsum @ r1, 0.0) @ r2
    rel = np.linalg.norm(out - exp) / np.linalg.norm(exp)
    print("self-test rel err:", rel)
